# revision 3
# baseline (speedup 1.0000x reference)
# ChildSumTreeLSTM on a complete binary tree (heap order), Trainium2 Bass.
#
# v2: fp8(e4m3) DoubleRow matmuls with interleaved per-bank accumulation
# chains at N=512, single merged sigmoid per gate (u-gate weights pre-doubled
# so one table/scale serves i,o,u), natural-child-order forget gates driven by
# a host-packed doubled-parent x copy, fused DVE ops, logits as a deferred
# tail phase, and log-softmax normalization on the host.
#
# Scaling scheme (psum = 2048 * z for every gate):
#   x stored fp8 (scale 1); ones-row = 16 at partition 44 of K-chunk 2
#   Wx stored fp8 * 2048 (u-rows *2); bias row = 128*b (u: 256*b)
#   h stored fp8 * 16;   Wh stored fp8 * 128 (u-rows *2)
#   activation SIG(psum/2048) -> i, o, sig(2z_u); tanh via 2*sig-1 on DVE
#   logits: (W_out/16) bf16 stationary x fp8 h (x16) -> exact-scale psum
import numpy as np
import ml_dtypes

E, H, L, DEPTH = 300, 256, 5, 17
NCORES = 8
CORE_DEPTH = 8                  # 8 local levels per core: 8192 .. 64
TILE = 512

BF16 = ml_dtypes.bfloat16
F8 = ml_dtypes.float8_e4m3      # TRN FP8_EXP4 (max 240)

SW = 2048.0                     # Wx scale (x scale 1)
SH = 16.0                       # h scale; Wh scale = SW / SH = 128


def _level_sizes(core_depth):
    return [1 << (13 - i) for i in range(core_depth)]  # leaf 8192 first


def _level_offsets(sizes):
    offs, o = [], 0
    for n in sizes:
        offs.append(o)
        o += n
    return offs, o


# ---------------------------------------------------------------------------
# Device kernel builder
# ---------------------------------------------------------------------------
_NC_CACHE = {}


def build_nc(core_depth=CORE_DEPTH, repeats=1, drop_xpd_dma=False):
    key = (core_depth, repeats, drop_xpd_dma)
    if key in _NC_CACHE:
        return _NC_CACHE[key]
    import concourse.bacc as bacc
    import concourse.mybir as mybir
    import concourse.tile as tile

    fp32 = mybir.dt.float32
    bf16 = mybir.dt.bfloat16
    f8 = mybir.dt.float8e4
    DR = mybir.MatmulPerfMode.DoubleRow
    SIG = mybir.ActivationFunctionType.Sigmoid
    MUL = mybir.AluOpType.mult
    ADD = mybir.AluOpType.add
    SUB = mybir.AluOpType.subtract

    sizes = _level_sizes(core_depth)
    offs, nloc = _level_offsets(sizes)
    nroot = sizes[-1]

    nc = bacc.Bacc("TRN2", target_bir_lowering=False, debug=False,
                   num_devices=NCORES)
    xk = nc.dram_tensor("xk", [128, 3 * nloc], f8, kind="ExternalInput")
    xpd = nc.dram_tensor("xpd", [128, 3 * nloc], f8, kind="ExternalInput")
    wx = nc.dram_tensor("wx", [128, 3 * 1024], f8, kind="ExternalInput")
    wh = nc.dram_tensor("wh", [128, 2 * 1024], f8, kind="ExternalInput")
    wo = nc.dram_tensor("wo", [128, 10], bf16, kind="ExternalInput")
    bout5 = nc.dram_tensor("bout5", [5, 1], fp32, kind="ExternalInput")
    out5 = nc.dram_tensor("out5", [5, nloc], bf16, kind="ExternalOutput")
    outhc = nc.dram_tensor("outhc", [128, 4 * nroot], fp32,
                           kind="ExternalOutput")

    xk_v = xk.ap().rearrange("p (k n) -> p k n", k=3)
    xpd_v = xpd.ap().rearrange("p (k n) -> p k n", k=3)
    wx_v = wx.ap().rearrange("p (k m) -> p k m", k=3)
    wh_v = wh.ap().rearrange("p (k m) -> p k m", k=2)

    with tile.TileContext(nc) as tc:
        with tc.tile_pool(name="wpool", bufs=1) as wpool, \
             tc.tile_pool(name="xpool", bufs=3) as xpool, \
             tc.tile_pool(name="gpool", bufs=2) as gpool, \
             tc.tile_pool(name="spool", bufs=2) as spool, \
             tc.tile_pool(name="stpool", bufs=1) as stpool:

            wx_sb = wpool.tile([128, 3, 1024], f8, tag="wx")
            wh_sb = wpool.tile([128, 2, 1024], f8, tag="wh")
            wo_sb = wpool.tile([128, 2, 5], bf16, tag="wo")
            bout5_sb = wpool.tile([5, 1], fp32, tag="bout5")
            nc.sync.dma_start(wx_sb[:], wx_v[:])
            nc.sync.dma_start(wh_sb[:], wh_v[:])
            nc.sync.dma_start(wo_sb[:], wo.ap().rearrange(
                "p (k m) -> p k m", k=2))
            nc.sync.dma_start(bout5_sb[:], bout5.ap())

            def body():
                h8_all = stpool.tile([128, 2, nloc], f8, tag="h8")
                outhc_sb = stpool.tile([128, 4 * nroot], fp32, tag="outhc")
                with tc.tile_pool(name="pz", bufs=1, space="PSUM") as pzp, \
                     tc.tile_pool(name="pf", bufs=1, space="PSUM") as pfp:
                    ct_all = [None] * core_depth
                    hsum_for = [None] * (core_depth + 1)
                    fcsum_for = [None] * (core_depth + 1)

                    def emit_w1(lvl, t):
                        n = sizes[lvl]
                        off = offs[lvl]
                        is_leaf = lvl == 0
                        is_root = lvl == core_depth - 1
                        n2 = n // 2
                        if t == 0:
                            ct_t = spool.tile([128, 2, n], bf16,
                                              tag="ct")
                            ct_all[lvl] = ct_t
                            if not is_root:
                                hs_t = spool.tile(
                                    [128, 2, max(n2, 1)], f8, tag="hsum")
                                hsum_for[lvl + 1] = hs_t
                        t0 = t * TILE
                        tn = min(TILE, n - t0)
                        xt = xpool.tile([128, 3, TILE], f8, tag="xt")
                        nc.sync.dma_start(
                            xt[:, :, :tn],
                            xk_v[:, :, off + t0: off + t0 + tn])
                        pz = pzp.tile([128, 6, TILE], fp32, tag="pz")
                        for s2 in range(6):
                            m0 = s2 * 128
                            nc.tensor.matmul(
                                pz[:, s2, :tn], wx_sb[:, 0:2, m0:m0 + 128],
                                xt[:, 0:2, :tn], start=True, stop=False,
                                perf_mode=DR)
                        for s2 in range(6):
                            m0 = s2 * 128
                            r0 = 64 * (s2 % 2)
                            nc.tensor.matmul(
                                pz[:, s2, :tn],
                                wx_sb[r0:r0 + 45, 2, m0:m0 + 128],
                                xt[r0:r0 + 45, 2, :tn],
                                start=False, stop=is_leaf,
                                tile_position=(r0, 0))
                        if not is_leaf:
                            for s2 in range(6):
                                m0 = s2 * 128
                                nc.tensor.matmul(
                                    pz[:, s2, :tn],
                                    wh_sb[:, 0:2, m0:m0 + 128],
                                    hsum_for[lvl][:, 0:2, t0:t0 + tn],
                                    start=False, stop=True, perf_mode=DR)
                        iou = gpool.tile([128, 6, TILE], bf16, tag="iou")
                        for g in range(3):
                            nc.scalar.activation(
                                iou[:, 2 * g:2 * g + 2, :tn],
                                pz[:, 2 * g:2 * g + 2, :tn], SIG,
                                scale=1.0 / SW)
                        t0v = gpool.tile([128, 2, TILE], bf16, tag="t0")
                        nc.vector.scalar_tensor_tensor(
                            t0v[:, :, :tn], iou[:, 4:6, :tn], 0.5,
                            iou[:, 0:2, :tn], op0=SUB, op1=MUL)
                        ctv = ct_all[lvl][:, :, t0:t0 + tn]
                        if is_leaf:
                            nc.vector.tensor_scalar(
                                ctv, t0v[:, :, :tn], 2.0, None, op0=MUL)
                        else:
                            nc.vector.scalar_tensor_tensor(
                                ctv, t0v[:, :, :tn], 2.0,
                                fcsum_for[lvl][:, :, t0:t0 + tn],
                                op0=MUL, op1=ADD)
                        cs = gpool.tile([128, 2, TILE], bf16, tag="cs")
                        nc.scalar.activation(cs[:, :, :tn], ctv, SIG,
                                             scale=2.0)
                        t1v = gpool.tile([128, 2, TILE], bf16, tag="t1")
                        nc.vector.scalar_tensor_tensor(
                            t1v[:, :, :tn], cs[:, :, :tn], 0.5,
                            iou[:, 2:4, :tn], op0=SUB, op1=MUL)
                        h8v = h8_all[:, :, off + t0: off + t0 + tn]
                        nc.vector.tensor_scalar(
                            h8v, t1v[:, :, :tn], 2.0 * SH, None, op0=MUL)
                        if not is_root:
                            q0 = t0 // 2
                            pn = tn // 2
                            hv = h8v.rearrange(
                                "p c (n two) -> p c n two", two=2)
                            nc.vector.tensor_add(
                                hsum_for[lvl + 1][:, :, q0:q0 + pn],
                                hv[:, :, :, 0], hv[:, :, :, 1])
                        else:
                            nc.vector.tensor_copy(
                                outhc_sb[:, 0:2 * nroot].rearrange(
                                    "p (c n) -> p c n", c=2), h8v)
                            nc.vector.tensor_copy(
                                outhc_sb[:, 2 * nroot:4 * nroot]
                                .rearrange("p (c n) -> p c n", c=2), ctv)

                    def emit_w2(lvl, t):
                        n = sizes[lvl]
                        off = offs[lvl]
                        n2 = n // 2
                        if t == 0:
                            fc_t = spool.tile(
                                [128, 2, max(n2, 1)], bf16, tag="fcsum")
                            fcsum_for[lvl + 1] = fc_t
                        t0 = t * TILE
                        tn = min(TILE, n - t0)
                        xp = xpool.tile([128, 3, TILE], f8, tag="xp")
                        nc.sync.dma_start(
                            xp[:, :, :tn],
                            xpd_v[:, :, off + t0: off + t0 + tn])
                        h8v = h8_all[:, :, off + t0: off + t0 + tn]
                        pf = pfp.tile([128, 2, TILE], fp32, tag="pf")
                        for c in range(2):
                            m0 = 768 + c * 128
                            nc.tensor.matmul(
                                pf[:, c, :tn],
                                wx_sb[:, 0:2, m0:m0 + 128],
                                xp[:, 0:2, :tn], start=True,
                                stop=False, perf_mode=DR)
                        for c in range(2):
                            m0 = 768 + c * 128
                            r0 = 64 * c
                            nc.tensor.matmul(
                                pf[:, c, :tn],
                                wx_sb[r0:r0 + 45, 2, m0:m0 + 128],
                                xp[r0:r0 + 45, 2, :tn],
                                start=False, stop=False,
                                tile_position=(r0, 0))
                        for c in range(2):
                            m0 = 768 + c * 128
                            nc.tensor.matmul(
                                pf[:, c, :tn],
                                wh_sb[:, 0:2, m0:m0 + 128],
                                h8v, start=False, stop=True,
                                perf_mode=DR)
                        ft = gpool.tile([128, 2, TILE], bf16, tag="ft")
                        for c in range(2):
                            nc.scalar.activation(
                                ft[:, c, :tn], pf[:, c, :tn], SIG,
                                scale=1.0 / SW)
                        fct = gpool.tile([128, 2, TILE], bf16, tag="fct")
                        nc.gpsimd.tensor_tensor(
                            fct[:, :, :tn], ft[:, :, :tn],
                            ct_all[lvl][:, :, t0:t0 + tn], MUL)
                        q0 = t0 // 2
                        pn = tn // 2
                        fv = fct[:, :, :tn].rearrange(
                            "p c (n two) -> p c n two", two=2)
                        nc.vector.tensor_add(
                            fcsum_for[lvl + 1][:, :, q0:q0 + pn],
                            fv[:, :, :, 0], fv[:, :, :, 1])

                    # software-pipelined emission: leaf wave1 first, then
                    # interleave level L's forget-gate wave with level L+1's
                    # wave1 (two w2 tiles complete one w1 tile's fc_sum)
                    for t in range((sizes[0] + TILE - 1) // TILE):
                        emit_w1(0, t)
                    for lvl in range(core_depth - 1):
                        nw2 = (sizes[lvl] + TILE - 1) // TILE
                        nw1n = (sizes[lvl + 1] + TILE - 1) // TILE
                        k = 0
                        for j in range(nw2):
                            emit_w2(lvl, j)
                            ready = ((j + 1) * TILE // 2) // TILE
                            while k < min(ready, nw1n):
                                emit_w1(lvl + 1, k)
                                k += 1
                        while k < nw1n:
                            emit_w1(lvl + 1, k)
                            k += 1
                # --- logits tail: wo (bf16) x h8, two banks + DVE combine ---
                with tc.tile_pool(name="pl", bufs=2, space="PSUM") as plp:
                    for t in range((nloc + TILE - 1) // TILE):
                        c0 = t * TILE
                        cn = min(TILE, nloc - c0)
                        pl = plp.tile([5, TILE], fp32, tag="pl")
                        for kc in range(2):
                            nc.tensor.matmul(
                                pl[:, :cn], wo_sb[:, kc, :],
                                h8_all[:, kc, c0:c0 + cn],
                                start=(kc == 0), stop=(kc == 1))
                        lo = gpool.tile([5, TILE], bf16, tag="lo")
                        nc.vector.tensor_scalar(
                            lo[:, :cn], pl[:, :cn], bout5_sb[:], None,
                            op0=ADD)
                        nc.sync.dma_start(out5.ap()[:, c0:c0 + cn],
                                          lo[:, :cn])
                nc.sync.dma_start(outhc.ap(), outhc_sb[:])

            if repeats == 1:
                body()
            else:
                engs = (mybir.EngineType.PE, mybir.EngineType.Activation,
                        mybir.EngineType.DVE, mybir.EngineType.SP,
                        mybir.EngineType.Pool)
                with tc.For_i(0, repeats, 1, hint_engines=engs):
                    body()
    nc.compile()
    _NC_CACHE[key] = nc
    return nc


# ---------------------------------------------------------------------------
# Host-side packing
# ---------------------------------------------------------------------------
def _core_node_index(core_depth=CORE_DEPTH, ncores=NCORES):
    per_core = []
    top = DEPTH - core_depth
    for k in range(ncores):
        parts = []
        for d in range(DEPTH - 1, top - 1, -1):
            s = (1 << d) - 1
            m = 1 << (d - 3)
            parts.append(np.arange(s + k * m, s + (k + 1) * m))
        per_core.append(np.concatenate(parts))
    return per_core


def _q8(a):
    return np.clip(a, -240.0, 240.0).astype(F8)


def _pack_weights(inp):
    f32 = np.float32
    Wx = np.vstack([inp["W_ix"], inp["W_ox"], inp["W_ux"], inp["W_fx"]])
    Wh = np.vstack([inp["W_ih"], inp["W_oh"], inp["W_uh"], inp["W_fh"]])
    b = np.concatenate([inp["b_ix"] + inp["b_ih"], inp["b_ox"] + inp["b_oh"],
                        inp["b_ux"] + inp["b_uh"], inp["b_fx"] + inp["b_fh"]])
    WxT = np.zeros((384, 1024), f32)
    WxT[:E] = SW * Wx.T
    WxT[E] = (SW / SH) * b                 # ones-row = 16 -> 2048*b total
    WxT[:, 512:768] *= 2.0                 # u-gate: fold SIG(2z)
    WxT[320:365] = WxT[256:301]            # tail copy for row-group pairing
    WhT = np.zeros((256, 1024), f32)
    WhT[:] = (SW / SH) * Wh.T
    WhT[:, 512:768] *= 2.0
    wxp = _q8(WxT).reshape(3, 128, 1024).transpose(1, 0, 2).reshape(128, -1)
    whp = _q8(WhT).reshape(2, 128, 1024).transpose(1, 0, 2).reshape(128, -1)
    WoT = np.zeros((256, 5), f32)
    WoT[:] = inp["W_out"].T / SH
    wop = WoT.astype(BF16).reshape(2, 128, 5).transpose(1, 0, 2).reshape(
        128, 10)
    return {
        "wx": wxp, "wh": whp, "wo": np.ascontiguousarray(wop),
        "bout5": np.ascontiguousarray(
            inp["b_out"].reshape(5, 1).astype(f32)),
    }


def _pack_x(x, idx, nloc):
    xTp = np.zeros((384, nloc), F8)
    xTp[:E] = _q8(np.asarray(x, np.float32)[idx].T)
    xTp[E] = F8(16.0)
    xTp[320:365] = xTp[256:301]            # tail copy for row-group pairing
    return np.ascontiguousarray(
        xTp.reshape(3, 128, nloc).transpose(1, 0, 2).reshape(128, 3 * nloc))


def _host_top(inp, h_roots, c_roots, core_depth=CORE_DEPTH):
    top = DEPTH - core_depth
    ntop = (1 << top) - 1
    x = np.asarray(inp["x"], np.float32)

    def sig(z):
        return 1.0 / (1.0 + np.exp(-z))

    h_sum = np.zeros((ntop, H), np.float32)
    fc_sum = np.zeros((ntop, H), np.float32)
    h_all = np.zeros((ntop, H), np.float32)
    ks = np.arange(h_roots.shape[0])
    g = ntop + ks
    p = (g - 1) // 2
    xf = x[p] @ inp["W_fx"].T + inp["b_fx"]
    f = sig(xf + h_roots @ inp["W_fh"].T + inp["b_fh"])
    np.add.at(h_sum, p, h_roots)
    np.add.at(fc_sum, p, f * c_roots)
    for d in range(top - 1, -1, -1):
        s, e = (1 << d) - 1, (1 << (d + 1)) - 1
        hs = h_sum[s:e]
        i = sig(x[s:e] @ inp["W_ix"].T + inp["b_ix"]
                + hs @ inp["W_ih"].T + inp["b_ih"])
        o = sig(x[s:e] @ inp["W_ox"].T + inp["b_ox"]
                + hs @ inp["W_oh"].T + inp["b_oh"])
        u = np.tanh(x[s:e] @ inp["W_ux"].T + inp["b_ux"]
                    + hs @ inp["W_uh"].T + inp["b_uh"])
        c = i * u + fc_sum[s:e]
        h = o * np.tanh(c)
        h_all[s:e] = h
        if d > 0:
            pp = (np.arange(s, e) - 1) // 2
            xf = x[pp] @ inp["W_fx"].T + inp["b_fx"]
            f = sig(xf + h @ inp["W_fh"].T + inp["b_fh"])
            np.add.at(h_sum, pp, h)
            np.add.at(fc_sum, pp, f * c)
    logits = h_all @ inp["W_out"].T + inp["b_out"]
    m = logits.max(-1, keepdims=True)
    lse = m + np.log(np.exp(logits - m).sum(-1, keepdims=True))
    return logits - lse


# ---------------------------------------------------------------------------
# Entry point
# ---------------------------------------------------------------------------
def kernel(**inputs):
    from concourse.bass_utils import run_bass_kernel_spmd

    inp = {k: np.asarray(v) for k, v in inputs.items()}
    sizes = _level_sizes(CORE_DEPTH)
    offs, nloc = _level_offsets(sizes)
    nroot = sizes[-1]
    nc = build_nc(CORE_DEPTH)

    w = _pack_weights(inp)
    idxs = _core_node_index()
    in_maps = []
    for k in range(NCORES):
        m = dict(w)
        m["xk"] = _pack_x(inp["x"], idxs[k], nloc)
        m["xpd"] = _pack_x(inp["x"], (idxs[k] - 1) // 2, nloc)
        in_maps.append(m)
    res = run_bass_kernel_spmd(nc, in_maps, list(range(NCORES)))

    N = inp["x"].shape[0]
    out = np.zeros((N, 5), np.float32)
    h_roots = np.zeros((NCORES * nroot, H), np.float32)
    c_roots = np.zeros((NCORES * nroot, H), np.float32)
    for k in range(NCORES):
        r = res.results[k]
        o5 = np.asarray(r["out5"], np.float32)      # [5, nloc] logits
        m = o5.max(0, keepdims=True)
        lse = m + np.log(np.exp(o5 - m).sum(0, keepdims=True))
        out[idxs[k]] = (o5 - lse).T
        hc = np.asarray(r["outhc"], np.float32)     # [128, 4*nroot]
        h = hc[:, 0:2 * nroot].reshape(128, 2, nroot) / SH
        c = hc[:, 2 * nroot:4 * nroot].reshape(128, 2, nroot)
        for j in range(nroot):
            h_roots[k * nroot + j] = h[:, :, j].T.reshape(-1)
            c_roots[k * nroot + j] = c[:, :, j].T.reshape(-1)
    top = DEPTH - CORE_DEPTH
    out[: (1 << top) - 1] = _host_top(inp, h_roots, c_roots)
    return out
